# revision 13
# baseline (speedup 1.0000x reference)
"""MLA (absorbed-weight multi-head latent attention) TRN2 Bass kernel, v2.

Problem: B=2, N=NKV=2048, E=4096, H=16, HD=256, LQ=512, LKV=256.
  C_q  = Q @ Wq_d                 [B,N,LQ]
  C_kv = K @ Wkv_d                [B,Nkv,LKV]
  CqWqk = (C_q @ W_qk)            [B,N,H,LKV]
  scores = einsum('bnhl,bkl->bhnk', CqWqk, C_kv) / sqrt(LKV)
  attn = softmax(scores, -1)
  V_up = (C_kv @ Wv_u)            [B,Nkv,H,HD]
  out  = einsum('bhnk,bkhd->bnhd', attn, V_up) -> [B,N,E]

Sharding: 8 cores = (batch b in 0..1) x (query quarter q in 0..3).
Each core handles n-rows [q*512,(q+1)*512) of batch b for ALL heads.

All matmul operands are bf16 (PSUM accumulation fp32); host pre-casts
and pre-transposes so the device needs no transposes:
  C_qT   [LQ, n]  = lhsT Wq_d   @ rhs Q^T      (contract E, streamed)
  C_kvT  [LKV, k] = lhsT Wkv_d  @ rhs K^T      (contract E, streamed)
  CqWqkT [LKV, n] = lhsT W_qk_h @ rhs C_qT     (contract LQ, all heads upfront)
  Vup_h  [k, HD]  = lhsT C_kvT  @ rhs Wv_u_h   (contract LKV)
  S^T    [k, n]   = lhsT C_kvT  @ rhs CqWqkT_h (contract LKV)
  P^T    = exp(S^T / 16)  (no max-subtraction: |S/16| <= ~6, safe)
  out    [n, HD+] = lhsT P^T    @ rhs [Vup_h | 1 1]  (contract k)
  out[:, :256] /= out[:, 256]  (ones-column row-sum denominator)
"""
import numpy as np

B, N, NKV, E, H = 2, 2048, 2048, 4096, 16
HD, LQ, LKV = 256, 512, 256
NCORES = 8
NQ = N // 4          # 512 query rows per core
ECH = E // 128       # 32 e-chunks
KCH = NKV // 128     # 16 k-chunks
NCK = NQ // 128      # 4 n-chunks per core

_cache = {}


def build_nc(iters=1, stop_after="full", fake_ckv=False):
    import concourse.bass as bass
    from concourse import bacc
    import concourse.mybir as mybir
    import concourse.tile as tile

    do_cqw = stop_after in ("cqw", "scores", "full")
    do_scores = stop_after in ("scores", "full")
    do_pv = stop_after == "full"
    dt = mybir.dt
    bf16 = dt.bfloat16
    fp8 = dt.float8e4
    f32 = dt.float32
    DR = mybir.MatmulPerfMode.DoubleRow

    nc = bacc.Bacc(None, target_bir_lowering=False)
    QT = nc.dram_tensor("QT", [E, NQ], bf16, kind="ExternalInput")
    KT = nc.dram_tensor("KT", [E, NKV], bf16, kind="ExternalInput")
    WQD = nc.dram_tensor("WQD", [E, LQ], bf16, kind="ExternalInput")
    WQK = nc.dram_tensor("WQK", [LQ, H * LKV], bf16, kind="ExternalInput")
    WKVD = nc.dram_tensor("WKVD", [E, LKV], bf16, kind="ExternalInput")
    WVU = nc.dram_tensor("WVU", [LKV, H * HD], bf16, kind="ExternalInput")
    OUT = nc.dram_tensor("OUT", [NQ, E], f32, kind="ExternalOutput")

    Exp = mybir.ActivationFunctionType.Exp

    with tile.TileContext(nc) as tc:
        with tc.tile_pool(name="persist", bufs=1) as persist, \
             tc.tile_pool(name="psA", bufs=1, space="PSUM") as psA, \
             tc.tile_pool(name="psB", bufs=2, space="PSUM") as psB, \
             tc.tile_pool(name="psC", bufs=2, space="PSUM") as psC, \
             tc.tile_pool(name="qtp", bufs=2) as qtp, \
             tc.tile_pool(name="ktp", bufs=4) as ktp, \
             tc.tile_pool(name="hp", bufs=2) as hp_pool, \
             tc.tile_pool(name="ptp", bufs=3) as ptp:
            loop_ctx = tc.For_i(0, iters, 1,
                                hint_engines=(mybir.EngineType.PE,)) \
                if iters > 1 else None
            if loop_ctx is not None:
                loop_ctx.__enter__()

            expbias = persist.tile([128, 1], f32)
            nc.vector.memset(expbias, -3.0)
            cqt = persist.tile([128, 4, NQ], bf16)        # C_qT  [LQ, n]
            ckvt = persist.tile([128, 2, NKV], bf16)      # C_kvT [LKV, k]
            cqwall = persist.tile([128, 2, H, NQ], bf16)  # CqWqkT all heads
            wqk_all = persist.tile([128, 4, H * LKV], bf16)
            wvu_all = persist.tile([128, 2, H * HD], bf16)

            # ---------- phase 1: C_qT (streamed 4-ec groups) ----------
            if True:
                accq = [psA.tile([128, 512], f32, tag=f"a{i}", name=f"accq{i}")
                        for i in range(4)]
                for g in range(8):
                    qt = qtp.tile([128, 4, NQ], bf16, tag="qt")
                    wqd = qtp.tile([128, 4, LQ], bf16, tag="wqd")
                    sl = slice(g * 512, (g + 1) * 512)
                    nc.sync.dma_start(
                        out=qt, in_=QT[sl, :].rearrange("(c p) n -> p c n", p=128))
                    nc.sync.dma_start(
                        out=wqd, in_=WQD[sl, :].rearrange("(c p) l -> p c l", p=128))
                    for ec in range(4):
                        for lc in range(4):
                            nc.tensor.matmul(
                                accq[lc], wqd[:, ec, lc * 128:(lc + 1) * 128],
                                qt[:, ec, :],
                                start=(g == 0 and ec == 0),
                                stop=(g == 7 and ec == 3))
                for lc in range(4):
                    if lc % 2 == 0:
                        nc.vector.tensor_copy(cqt[:, lc, :], accq[lc])
                    else:
                        nc.scalar.copy(cqt[:, lc, :], accq[lc])

            # ---------- phase 2: C_kvT (streamed KT) ----------
            if True:
                accs = [psA.tile([128, 512], f32, tag=f"a{i}", name=f"acck{i}")
                        for i in range(4)]
                accs += [psB.tile([128, 512], f32, tag="sw", name=f"acck{4+i}")
                         for i in range(2)]
                accs += [psC.tile([128, 512], f32, tag="v", name=f"acck{6+i}")
                         for i in range(2)]
                for ec in range(ECH):
                    ktt = ktp.tile([128, NKV], bf16, tag="kt")
                    nc.sync.dma_start(out=ktt, in_=KT[ec * 128:(ec + 1) * 128, :])
                    if ec in (14, 17, 20, 23):
                        # preload weights needed from phase 2.5 onward, split
                        # into chunks so the QT/KT streams are not starved
                        i4 = (ec - 14) // 3
                        nc.sync.dma_start(
                            out=wqk_all[:, i4, :],
                            in_=WQK[i4 * 128:(i4 + 1) * 128, :])
                    if ec in (26, 29):
                        i2 = (ec - 26) // 3
                        nc.sync.dma_start(
                            out=wvu_all[:, i2, :],
                            in_=WVU[i2 * 128:(i2 + 1) * 128, :])
                    if ec % 4 == 0:
                        wkvd_t = ktp.tile([128, 4, LKV], bf16, tag="wkvd")
                        nc.sync.dma_start(
                            out=wkvd_t,
                            in_=WKVD[ec * 128:(ec + 4) * 128, :]
                            .rearrange("(c p) l -> p c l", p=128))
                    for lc in range(2):
                        for nt in range(4):
                            nc.tensor.matmul(
                                accs[lc * 4 + nt],
                                wkvd_t[:, ec % 4, lc * 128:(lc + 1) * 128],
                                ktt[:, nt * 512:(nt + 1) * 512],
                                start=(ec == 0), stop=(ec == ECH - 1))
                for lc in range(2):
                    for nt in range(4):
                        dst = ckvt[:, lc, nt * 512:(nt + 1) * 512]
                        if nt % 2 == 0:
                            nc.vector.tensor_copy(dst, accs[lc * 4 + nt])
                        else:
                            nc.scalar.copy(dst, accs[lc * 4 + nt])

            # ---------- phase 2.5: CqWqkT for ALL heads ----------
            for h in range(H if do_cqw else 0):
                for lkc in range(2):
                    ps = psB.tile([128, 512], f32, tag="sw")
                    base = h * LKV + lkc * 128
                    for lc in range(4):
                        nc.tensor.matmul(
                            ps, wqk_all[:, lc, base:base + 128],
                            cqt[:, lc, :], start=(lc == 0), stop=(lc == 3))
                    if lkc == 0:
                        nc.vector.tensor_copy(cqwall[:, lkc, h, :], ps)
                    else:
                        nc.scalar.copy(cqwall[:, lkc, h, :], ps)

            # ---------- phase 3: attention, heads processed in pairs ----------
            if True:
                for hp in range(H // 2 if do_scores else 0):
                    # V_up for BOTH heads of the pair in one 512-wide matmul
                    # per (kc, lkc); fp8 rows strided 272 per head
                    vup = hp_pool.tile([128, KCH, 2, 272], fp8, tag="vup")
                    nc.vector.memset(vup[:, :, :, 256:258], 1.0)
                    for kc in range(KCH):
                        psv = psC.tile([128, 512], f32, tag="v")
                        for lkc in range(2):
                            nc.tensor.matmul(
                                psv, ckvt[:, lkc, kc * 128:(kc + 1) * 128],
                                wvu_all[:, lkc, hp * 512:(hp + 1) * 512],
                                start=(lkc == 0), stop=(lkc == 1))
                        nc.vector.tensor_copy(
                            vup[:, kc, :, 0:256],
                            psv.rearrange("p (j d) -> p j d", j=2))
                    for j in range(2):
                        h = 2 * hp + j
                        pso = [psA.tile([128, 258], f32, tag=f"a{i}",
                                        name=f"pso{i}") for i in range(NCK)] \
                            if do_pv else None
                        ptall = ptp.tile([128, KCH // 2, 2, NQ], fp8, tag="pt")
                        for kc in range(KCH):
                            pss = psB.tile([128, 512], f32, tag="sw")
                            for lkc in range(2):
                                nc.tensor.matmul(
                                    pss, ckvt[:, lkc, kc * 128:(kc + 1) * 128],
                                    cqwall[:, lkc, h, :],
                                    start=(lkc == 0), stop=(lkc == 1))
                            nc.scalar.activation(
                                out=ptall[:, kc // 2, kc % 2, :], in_=pss,
                                func=Exp, scale=1.0 / 16.0, bias=expbias)
                        if do_pv:
                            for kcp in range(KCH // 2):
                                for nk in range(NCK):
                                    nc.tensor.matmul(
                                        pso[nk],
                                        ptall[:, kcp, :, nk * 128:(nk + 1) * 128],
                                        vup[:, 2 * kcp:2 * kcp + 2, j, 0:258],
                                        start=(kcp == 0),
                                        stop=(kcp == KCH // 2 - 1),
                                        perf_mode=DR)

                        # normalize + store (one batched DMA per head)
                        if do_pv:
                            ot = ptp.tile([128, NCK, HD], f32, tag="ot")
                            for nk in range(NCK):
                                den = hp_pool.tile([128, 1], f32, tag="den")
                                nc.vector.reciprocal(den, pso[nk][:, 256:257])
                                nc.vector.tensor_scalar_mul(
                                    ot[:, nk, :], pso[nk][:, 0:256], den)
                            nc.sync.dma_start(
                                out=OUT.rearrange("(c p) e -> p c e", p=128)
                                [:, :, h * HD:(h + 1) * HD],
                                in_=ot)
            if not do_pv:
                dummy = ptp.tile([128, NCK, HD], f32, tag="ot")
                nc.vector.memset(dummy, 0.5)
                nc.sync.dma_start(
                    out=OUT.rearrange("(c p) e -> p c e", p=128)[:, :, 0:HD],
                    in_=dummy)
            if loop_ctx is not None:
                loop_ctx.__exit__(None, None, None)

    nc.finalize()
    return nc


def get_nc(iters=1, stop_after="full", fake_ckv=False):
    key = (iters, stop_after, fake_ckv)
    if key not in _cache:
        _cache[key] = build_nc(iters, stop_after, fake_ckv)
    return _cache[key]


def make_in_maps(Q, K, Wq_d, W_qk, Wkv_d, Wv_u):
    import ml_dtypes
    bf = ml_dtypes.bfloat16
    Q = np.asarray(Q, dtype=np.float32)
    K = np.asarray(K, dtype=np.float32)
    weights = {
        "WQD": np.ascontiguousarray(np.asarray(Wq_d).astype(bf)),
        "WQK": np.ascontiguousarray(np.asarray(W_qk).astype(bf)),
        "WKVD": np.ascontiguousarray(np.asarray(Wkv_d).astype(bf)),
        "WVU": np.ascontiguousarray(np.asarray(Wv_u).astype(bf)),
    }
    kts = [np.ascontiguousarray(K[b].T.astype(bf)) for b in range(B)]
    qts = [Q[b].T.astype(bf) for b in range(B)]
    in_maps = []
    for c in range(NCORES):
        b, q = divmod(c, 4)
        m = dict(weights)
        m["KT"] = kts[b]
        m["QT"] = np.ascontiguousarray(qts[b][:, q * NQ:(q + 1) * NQ])
        in_maps.append(m)
    return in_maps


def kernel(Q, K, Wq_d, W_qk, Wkv_d, Wv_u):
    from concourse.bass_utils import run_bass_kernel_spmd

    nc = get_nc(1)
    in_maps = make_in_maps(Q, K, Wq_d, W_qk, Wkv_d, Wv_u)
    res = run_bass_kernel_spmd(nc, in_maps, core_ids=list(range(NCORES)))
    out = np.empty((B, N, E), dtype=np.float32)
    for c in range(NCORES):
        b, q = divmod(c, 4)
        out[b, q * NQ:(q + 1) * NQ, :] = res.results[c]["OUT"]
    return out
